# revision 1
# baseline (speedup 1.0000x reference)
"""CQT extractor kernel for Trainium2 (8 NeuronCores, data-parallel over batch).

Pipeline per core (2 audio rows):
  STFT-as-matmul with Hermitian folding (1024-long contraction instead of
  2048), magnitude via ACT Square/Sqrt, CQT projection matmul, log10.

Host side does only data movement (reflect pad, chunk-reversed copy for the
fold) and constant table generation; all FLOPs run on device.
"""

import math
from contextlib import ExitStack

import numpy as np


import concourse.tile as tile
from concourse import bacc, mybir
from concourse.bass_utils import run_bass_kernel_spmd
from concourse.masks import make_identity

# ---- problem constants (hardcoded per contest rules) ----
B = 16
L = 1310720
SR = 22050
HOP = 512
NFFT = 2048
NBINS = 84
BPO = 12
FMIN = 27.5

NF = 1 + L // HOP            # 2561 frames
PAD = NFFT // 2              # 1024
LP = L + 2 * PAD             # 1312768 reflect-padded length

NCORES = 8
ROWS_PER_CORE = B // NCORES  # 2

# frame tiling: 6 uniform tiles of 428 frames (fp32r needs even moving dim);
# frames past NF-1 are computed on zero padding and never written out
T_SIZES = [428] * 6
T_STARTS = [428 * i for i in range(6)]
T_ALLOC = 428

NGRP = 14                    # 128-chunk transpose groups per frame tile
WCH = NGRP * 128             # 1792 chunks staged per frame tile
NCH_PAD = 4 * T_STARTS[-1] + WCH + 1   # chunks incl. zero pad (+1 for +1 shift)
PADLEN = 128 * NCH_PAD

F32 = mybir.dt.float32
F32R = mybir.dt.float32r
LOG10E = 1.0 / math.log(10.0)


def _host_tables():
    """Folded DFT matrices and CQT weights, float64 -> float32."""
    j = np.arange(1024)
    n = (j + 1).astype(np.float64)          # contraction index j <-> sample n=j+1
    win = 0.5 * (1.0 - np.cos(2.0 * np.pi * n / NFFT))
    ang = 2.0 * np.pi * np.outer(n, np.arange(1024, dtype=np.float64)) / NFFT
    wc = win[:, None] * np.cos(ang)
    ws = win[:, None] * np.sin(ang)
    wc[1023] *= 0.5           # n=1024 term is double-counted by the fold
    ws[1023] = 0.0
    sf = np.fft.rfftfreq(NFFT, 1.0 / SR)[:1024]
    cf = FMIN * 2.0 ** (np.arange(NBINS, dtype=np.float64) / BPO)
    wq = np.exp(-np.abs(sf[:, None] - cf[None, :]) / (0.1 * cf[None, :]))
    return (
        np.ascontiguousarray(wc, dtype=np.float32),
        np.ascontiguousarray(ws, dtype=np.float32),
        np.ascontiguousarray(wq, dtype=np.float32),
    )


def _build_program():
    nc = bacc.Bacc("TRN2", target_bir_lowering=False, debug=False,
                   num_devices=NCORES)
    xp = nc.dram_tensor("xp", [ROWS_PER_CORE, PADLEN], F32R,
                        kind="ExternalInput").ap()
    zp = nc.dram_tensor("zp", [ROWS_PER_CORE, PADLEN], F32R,
                        kind="ExternalInput").ap()
    wc = nc.dram_tensor("wc", [8, 8, 128, 128], F32R, kind="ExternalInput").ap()
    ws = nc.dram_tensor("ws", [8, 8, 128, 128], F32R, kind="ExternalInput").ap()
    wq = nc.dram_tensor("wq", [1024, NBINS], F32R, kind="ExternalInput").ap()
    out = nc.dram_tensor("out", [ROWS_PER_CORE, NBINS, NF], F32,
                         kind="ExternalOutput").ap()

    with tile.TileContext(nc) as tc:
        with ExitStack() as ctx:
            _emit(ctx, tc, xp, zp, wc, ws, wq, out)
    nc.compile()
    return nc


def _emit(ctx, tc, xp, zp, wc, ws, wq, out):
    nc = tc.nc
    SQ = mybir.ActivationFunctionType.Square
    SQRT = mybir.ActivationFunctionType.Sqrt
    LN = mybir.ActivationFunctionType.Ln

    consts = ctx.enter_context(tc.tile_pool(name="consts", bufs=1))
    natp = ctx.enter_context(tc.tile_pool(name="natp", bufs=6))
    stage = ctx.enter_context(tc.tile_pool(name="stage", bufs=2))
    eo = ctx.enter_context(tc.tile_pool(name="eo", bufs=2))
    magp = ctx.enter_context(tc.tile_pool(name="magp", bufs=2))
    sqp = ctx.enter_context(tc.tile_pool(name="sqp", bufs=3))
    outp = ctx.enter_context(tc.tile_pool(name="outp", bufs=2))
    ps_mm = ctx.enter_context(tc.tile_pool(name="ps_mm", bufs=5, space="PSUM"))
    ps_tp = ctx.enter_context(tc.tile_pool(name="ps_tp", bufs=2, space="PSUM"))
    ps_cq = ctx.enter_context(tc.tile_pool(name="ps_cq", bufs=1, space="PSUM"))

    # constants (staged f32 -> rounded f32r copies)
    # [p, i_colblock, a_ktile, f] so each 512KB W-block DMA is contiguous
    wc_sb = consts.tile([128, 8, 8, 128], F32R, tag="wc_sb")
    ws_sb = consts.tile([128, 8, 8, 128], F32R, tag="ws_sb")
    wq_sb = consts.tile([128, 8, NBINS], F32R, tag="wq_sb")
    # direct f32r DMA of host-preblocked W, one 512KB DMA per column block,
    # pair-0 weights land first
    for i in range(8):
        nc.gpsimd.dma_start(wc_sb[:, i], wc[i].rearrange("a p f -> p a f"))
        nc.scalar.dma_start(ws_sb[:, i], ws[i].rearrange("a p f -> p a f"))
    nc.sync.dma_start(wq_sb[:], wq.rearrange("(a p) k -> p a k", a=8))
    ident = consts.tile([128, 128], F32, tag="ident")
    make_identity(nc, ident[:])
    identr = consts.tile([128, 128], F32R, tag="identr")
    nc.vector.tensor_copy(identr[:], ident[:])
    lnbias = consts.tile([128, 1], F32, tag="lnbias")
    nc.gpsimd.memset(lnbias[:], 1e-10)

    stage_count = [0]

    def emit_stage(r, it):
        """DMA + PE transpose + copyback + fold adds for one frame tile."""
        # during startup the scalar queue carries the W sin tables; route the
        # first two tiles' z loads through sync instead
        zq = nc.sync if stage_count[0] < 2 else nc.scalar
        stage_count[0] += 1
        T = T_SIZES[it]
        f0 = T_STARTS[it]
        cbase = 4 * f0
        Q = WCH // 4
        dts = stage.tile([128, 4, Q], F32, tag="dts")
        rev = stage.tile([128, 4, Q], F32, tag="rev")
        for g in range(NGRP):
            off = (cbase + 128 * g) * 128
            natx = natp.tile([128, 128], F32R, tag="natx")
            nc.sync.dma_start(
                natx[:],
                xp[r, off + 1: off + 1 + 128 * 128].rearrange(
                    "(c s) -> c s", s=128),
            )
            tpx = ps_tp.tile([128, 128], F32R, tag="tp")
            nc.tensor.transpose(tpx[:], natx[:], identr[:])
            nc.vector.tensor_copy(dts[:, :, 32 * g: 32 * (g + 1)],
                                  tpx.rearrange("p (q a) -> p a q", a=4))

            natz = natp.tile([128, 128], F32R, tag="natz")
            zq.dma_start(
                natz[:],
                zp[r, off: off + 128 * 128].rearrange("(c s) -> c s", s=128),
            )
            tpz = ps_tp.tile([128, 128], F32R, tag="tp")
            nc.tensor.transpose(tpz[:], natz[:], identr[:])
            nc.vector.tensor_copy(rev[:, :, 32 * g: 32 * (g + 1)],
                                  tpz.rearrange("p (q a) -> p a q", a=4))

        # folded operands: E[j,t]=x[512t+j+1]+x[512t+2047-j], O = diff
        # E term chunk c=4t+a -> phase a%4, q=t+a//4 (contiguous reads);
        # partner chunk c=4t+15-a -> phase (15-a)%4, q=t+(15-a)//4
        e4 = eo.tile([128, 8, T_ALLOC], F32R, tag="e4")
        o4 = eo.tile([128, 8, T_ALLOC], F32R, tag="o4")
        for a in range(8):
            d_ap = dts[:, a % 4, a // 4: a // 4 + T]
            r_ap = rev[:, (15 - a) % 4, (15 - a) // 4: (15 - a) // 4 + T]
            nc.vector.tensor_add(e4[:, a, :T], d_ap, r_ap)
            nc.gpsimd.tensor_sub(o4[:, a, :T], d_ap, r_ap)
        return e4, o4

    def emit_dft(r, it, e4, o4):
        """DFT matmuls + magnitude for one frame tile."""
        T = T_SIZES[it]
        mag = magp.tile([128, 8, T_ALLOC], F32R, tag="mag")
        for i in range(8):
            ps_re = ps_mm.tile([128, T_ALLOC], F32, tag="mm")
            for a in range(8):
                nc.tensor.matmul(
                    ps_re[:, :T],
                    wc_sb[:, i, a],
                    e4[:, a, :T],
                    start=(a == 0), stop=(a == 7),
                )
            ps_im = ps_mm.tile([128, T_ALLOC], F32, tag="mm")
            for a in range(8):
                nc.tensor.matmul(
                    ps_im[:, :T],
                    ws_sb[:, i, a],
                    o4[:, a, :T],
                    start=(a == 0), stop=(a == 7),
                )
            sq = sqp.tile([128, T_ALLOC], F32, tag="sq")
            nc.scalar.activation(sq[:, :T], ps_re[:, :T], SQ)
            sq2 = sqp.tile([128, T_ALLOC], F32, tag="sq2")
            nc.scalar.activation(sq2[:, :T], ps_im[:, :T], SQ)
            nc.vector.tensor_add(sq[:, :T], sq[:, :T], sq2[:, :T])
            nc.scalar.activation(mag[:, i, :T], sq[:, :T], SQRT)
        return mag

    def emit_cqt(r, it, mag):
        """CQT projection, log10, store."""
        T = T_SIZES[it]
        f0 = T_STARTS[it]
        ps_c = ps_cq.tile([NBINS, T_ALLOC], F32, tag="ps_c")
        for i in range(8):
            nc.tensor.matmul(
                ps_c[:, :T],
                wq_sb[:, i, :],
                mag[:, i, :T],
                start=(i == 0), stop=(i == 7),
            )
        V = min(T, NF - f0)          # valid (non-garbage) frames
        outt = outp.tile([NBINS, T_ALLOC], F32, tag="outt")
        nc.scalar.activation(outt[:, :V], ps_c[:, :V], LN,
                             bias=lnbias[:NBINS])
        nc.vector.tensor_scalar_mul(outt[:, :V], outt[:, :V], LOG10E)
        nc.sync.dma_start(out[r, :, f0: f0 + V], outt[:, :V])

    # software pipeline: PE order per slot is [transposes k+1][cqt k-1][dft k]
    # so the magnitude drain of tile k-1 and fold adds of k+1 hide under PE work
    tiles = [(r, it) for r in range(ROWS_PER_CORE) for it in range(6)]
    staged = emit_stage(*tiles[0])
    pending = None          # (r, it, mag) awaiting cqt
    for k, (r, it) in enumerate(tiles):
        nxt = emit_stage(*tiles[k + 1]) if k + 1 < len(tiles) else None
        if pending is not None:
            emit_cqt(*pending)
        mag = emit_dft(r, it, *staged)
        pending = (r, it, mag)
        staged = nxt
    emit_cqt(*pending)


_PROGRAM_CACHE = {}


def _get_program():
    if "nc" not in _PROGRAM_CACHE:
        _PROGRAM_CACHE["nc"] = _build_program()
    return _PROGRAM_CACHE["nc"]


def kernel(audio):
    audio = np.asarray(audio, dtype=np.float32)
    assert audio.shape == (B, L), audio.shape

    # host data movement: reflect pad + zero pad + within-chunk-reversed copy
    xpad = np.zeros((B, PADLEN), dtype=np.float32)
    xpad[:, :LP] = np.pad(audio, ((0, 0), (PAD, PAD)), mode="reflect")
    z = np.ascontiguousarray(
        xpad.reshape(B, NCH_PAD, 128)[:, :, ::-1]).reshape(B, PADLEN)

    wc, ws, wq = _host_tables()
    # (8_i, 8_a, 128_p, 128_f) blocks: wcb[i,a,p,f] = wc[128a+p, 128i+f]
    wc = np.ascontiguousarray(
        wc.reshape(8, 128, 8, 128).transpose(2, 0, 1, 3))
    ws = np.ascontiguousarray(
        ws.reshape(8, 128, 8, 128).transpose(2, 0, 1, 3))
    nc = _get_program()

    in_maps = []
    for c in range(NCORES):
        rows = slice(ROWS_PER_CORE * c, ROWS_PER_CORE * (c + 1))
        in_maps.append({
            "xp": np.ascontiguousarray(xpad[rows]),
            "zp": np.ascontiguousarray(z[rows]),
            "wc": wc, "ws": ws, "wq": wq,
        })

    res = run_bass_kernel_spmd(nc, in_maps, core_ids=list(range(NCORES)))
    out = np.concatenate([res.results[c]["out"] for c in range(NCORES)], axis=0)
    return np.ascontiguousarray(out, dtype=np.float32)



# revision 5
# speedup vs baseline: 1.9992x; 1.9992x over previous
"""CQT extractor kernel for Trainium2 (8 NeuronCores, data-parallel over batch).

Per core (2 audio rows): unfolded windowed-DFT as matmul against the raw
transposed audio, mixed precision — bf16 matmuls for frequency bins 0..127
(where single-bin CQT outputs need accuracy), fp8e4m3 DoubleRow matmuls for
bins 128..447 (CQT bins there aggregate many magnitudes, so fp8 noise
averages out).  Frequency bins >= 448 carry negligible CQT weight and are
truncated.  Magnitude via ACT Square / DVE mul / ACT Sqrt, CQT projection in
bf16, log10 batched per row on ACT (Ln + Copy-with-scale, so only one
activation-table switch per row).

Host side does only data movement (reflect pad, transpose to a phase-major
[128, 4, nq] chunk layout, dtype casts) and constant table generation; all
FLOPs run on device.  There are no on-chip transposes or fold adds: the DFT
reads the DMA-landed audio directly.
"""

import math
from contextlib import ExitStack

import ml_dtypes
import numpy as np

import concourse.tile as tile
from concourse import bacc, mybir
from concourse.bass_utils import run_bass_kernel_spmd

# ---- problem constants (hardcoded per contest rules) ----
B = 16
L = 1310720
SR = 22050
HOP = 512
NFFT = 2048
NBINS = 84
BPO = 12
FMIN = 27.5

NF = 1 + L // HOP            # 2561 frames
PAD = NFFT // 2              # 1024
LP = L + 2 * PAD             # 1312768 reflect-padded length

NCORES = 8
ROWS_PER_CORE = B // NCORES  # 2

FP = 448                     # frequency bins kept (of 1025)
FP_LO = 128                  # bins 0..127 in bf16
FP_HI = FP - FP_LO           # bins 128..447 in fp8 DoubleRow

# frame tiling: 6 uniform tiles of 428 frames; frames past NF-1 are computed
# on zero padding and never written out
NTILES = 6
T = 428
HT = T // 2                  # DoubleRow moving free dim is 2*HT <= 512

# phase-major audio layout: chunk c = 4*q + ph, sample x1[128*c + p]
NQ = 2576                    # q slots (covers chunk 4*2563+15 plus margin)

F32 = mybir.dt.float32
BF16 = mybir.dt.bfloat16
F8 = mybir.dt.float8e4
LOG10E = 1.0 / math.log(10.0)
DR = mybir.MatmulPerfMode.DoubleRow


def _host_tables():
    """Unfolded windowed DFT matrices (2048 x FP) and CQT weights, f64."""
    n = np.arange(1, 2048, dtype=np.float64)   # contraction j <-> sample n=j+1
    win = 0.5 * (1.0 - np.cos(2.0 * np.pi * n / NFFT))
    f = np.arange(FP, dtype=np.float64)
    ang = 2.0 * np.pi * np.outer(n, f) / NFFT
    wc = np.zeros((2048, FP))
    ws = np.zeros((2048, FP))
    wc[:2047] = win[:, None] * np.cos(ang)
    ws[:2047] = win[:, None] * np.sin(ang)
    sf = np.fft.rfftfreq(NFFT, 1.0 / SR)[:FP]
    cf = FMIN * 2.0 ** (np.arange(NBINS, dtype=np.float64) / BPO)
    wq = np.exp(-np.abs(sf[:, None] - cf[None, :]) / (0.1 * cf[None, :]))
    return wc, ws, wq


def _build_program():
    nc = bacc.Bacc("TRN2", target_bir_lowering=False, debug=False,
                   num_devices=NCORES)
    a16 = nc.dram_tensor("a16", [ROWS_PER_CORE, 128, 4, NQ], BF16,
                         kind="ExternalInput").ap()
    a8 = nc.dram_tensor("a8", [ROWS_PER_CORE, 128, 4, NQ], F8,
                        kind="ExternalInput").ap()
    wc16 = nc.dram_tensor("wc16", [128, 16, FP_LO], BF16,
                          kind="ExternalInput").ap()
    ws16 = nc.dram_tensor("ws16", [128, 16, FP_LO], BF16,
                          kind="ExternalInput").ap()
    wc8 = nc.dram_tensor("wc8", [128, 8, 2, FP_HI], F8,
                         kind="ExternalInput").ap()
    ws8 = nc.dram_tensor("ws8", [128, 8, 2, FP_HI], F8,
                         kind="ExternalInput").ap()
    wq = nc.dram_tensor("wq", [128, 4, NBINS], BF16, kind="ExternalInput").ap()
    out = nc.dram_tensor("out", [ROWS_PER_CORE, NBINS, NF], F32,
                         kind="ExternalOutput").ap()

    with tile.TileContext(nc) as tc:
        with ExitStack() as ctx:
            _emit(ctx, tc, a16, a8, wc16, ws16, wc8, ws8, wq, out)
    nc.compile()
    return nc


def _emit(ctx, tc, a16, a8, wc16, ws16, wc8, ws8, wq, out):
    nc = tc.nc
    SQ = mybir.ActivationFunctionType.Square
    SQRT = mybir.ActivationFunctionType.Sqrt
    LN = mybir.ActivationFunctionType.Ln

    consts = ctx.enter_context(tc.tile_pool(name="consts", bufs=1))
    a16p = ctx.enter_context(tc.tile_pool(name="a16p", bufs=3))
    a8p = ctx.enter_context(tc.tile_pool(name="a8p", bufs=3))
    sqp = ctx.enter_context(tc.tile_pool(name="sqp", bufs=6))
    magp = ctx.enter_context(tc.tile_pool(name="magp", bufs=3))
    cqp = ctx.enter_context(tc.tile_pool(name="cqp", bufs=2))
    outp = ctx.enter_context(tc.tile_pool(name="outp", bufs=2))
    ps_mm = ctx.enter_context(tc.tile_pool(name="ps_mm", bufs=5, space="PSUM"))
    ps_cq = ctx.enter_context(tc.tile_pool(name="ps_cq", bufs=2, space="PSUM"))

    # constants
    wc16_sb = consts.tile([128, 16, FP_LO], BF16, tag="wc16")
    ws16_sb = consts.tile([128, 16, FP_LO], BF16, tag="ws16")
    wc8_sb = consts.tile([128, 8, 2, FP_HI], F8, tag="wc8")
    ws8_sb = consts.tile([128, 8, 2, FP_HI], F8, tag="ws8")
    wq_sb = consts.tile([128, 4, NBINS], BF16, tag="wq")
    nc.sync.dma_start(wc16_sb[:], wc16)
    nc.scalar.dma_start(ws16_sb[:], ws16)
    nc.gpsimd.dma_start(wc8_sb[:], wc8)
    nc.scalar.dma_start(ws8_sb[:], ws8)
    nc.sync.dma_start(wq_sb[:], wq)
    lnbias = consts.tile([128, 1], F32, tag="lnbias")
    nc.gpsimd.memset(lnbias[:], 1e-10)

    def emit_tile(r, k):
        """DMA + DFT matmuls + magnitude for one frame tile; returns mag."""
        q0 = T * k
        a16_t = a16p.tile([128, 4, T + 4], BF16, tag="a16t")
        nc.sync.dma_start(a16_t[:], a16[r, :, :, q0: q0 + T + 4])
        a8_t = a8p.tile([128, 4, T + 4], F8, tag="a8t")
        nc.gpsimd.dma_start(a8_t[:], a8[r, :, :, q0: q0 + T + 4])

        mag = magp.tile([128, 4, T], BF16, tag="mag")
        for blk in range(4):
            W = FP_LO if blk == 0 else (128 if blk < 3 else 64)
            ps_re = ps_mm.tile([128, T], F32, tag="mm")
            ps_im = ps_mm.tile([128, T], F32, tag="mm")
            if blk == 0:
                for a in range(16):
                    rhs = a16_t[:, a % 4, a // 4: a // 4 + T]
                    nc.tensor.matmul(ps_re[:, :T], wc16_sb[:, a], rhs,
                                     start=(a == 0), stop=(a == 15))
                for a in range(16):
                    rhs = a16_t[:, a % 4, a // 4: a // 4 + T]
                    nc.tensor.matmul(ps_im[:, :T], ws16_sb[:, a], rhs,
                                     start=(a == 0), stop=(a == 15))
            else:
                fb = 128 * (blk - 1)
                for s in range(8):
                    ph = 2 * (s % 2)
                    qo = s // 2
                    for ps, wgt in ((ps_re, wc8_sb), (ps_im, ws8_sb)):
                        for h in range(2):
                            rhs = a8_t[:, ph: ph + 2,
                                       qo + h * HT: qo + h * HT + HT]
                            nc.tensor.matmul(
                                ps[:W, h * HT: (h + 1) * HT],
                                wgt[:, s, :, fb: fb + W],
                                rhs,
                                start=(s == 0), stop=(s == 7),
                                perf_mode=DR, skip_group_check=True,
                            )
            sq = sqp.tile([128, T], BF16, tag="sq")
            nc.scalar.activation(sq[:W, :], ps_re[:W, :T], SQ)
            sq2 = sqp.tile([128, T], BF16, tag="sq2")
            nc.scalar.activation(sq2[:W, :], ps_im[:W, :T], SQ)
            ss = sqp.tile([128, T], BF16, tag="ss")
            nc.vector.tensor_add(ss[:W, :], sq[:W, :], sq2[:W, :])
            nc.scalar.activation(mag[:W, blk], ss[:W, :], SQRT)
        return mag

    def emit_cqt(r, k, mag, cq_row):
        """CQT projection into PSUM, copy into the row accumulator."""
        q0 = T * k
        ps_c = ps_cq.tile([NBINS, T], F32, tag="cq")
        for a in range(3):
            nc.tensor.matmul(ps_c[:, :T], wq_sb[:, a], mag[:, a],
                             start=(a == 0), stop=False)
        nc.tensor.matmul(ps_c[:, :T], wq_sb[:64, 3], mag[:64, 3],
                         start=False, stop=True)
        V = min(T, NF - q0)
        nc.vector.tensor_copy(cq_row[:, q0: q0 + V], ps_c[:, :V])

    def emit_row_end(r, cq_row):
        """log10 = Ln then Copy-with-scale, then store the row."""
        out_t = outp.tile([NBINS, NF], F32, tag="outt")
        nc.scalar.activation(out_t[:], cq_row[:], LN, bias=lnbias[:NBINS])
        nc.scalar.mul(out_t[:], out_t[:], LOG10E)
        nc.sync.dma_start(out[r], out_t[:])

    # software pipeline: PE order per slot is [dft k][cqt k-1] so the
    # magnitude drain of tile k-1 hides under tile k's matmuls
    tiles = [(r, k) for r in range(ROWS_PER_CORE) for k in range(NTILES)]
    cq_rows = {r: cqp.tile([NBINS, NF], F32, tag="cqrow", name=f"cqrow{r}")
               for r in range(ROWS_PER_CORE)}
    pending = None
    row_done = None
    for r, k in tiles:
        mag = emit_tile(r, k)
        if pending is not None:
            pr, pk, pmag = pending
            emit_cqt(pr, pk, pmag, cq_rows[pr])
            if pk == NTILES - 1:
                row_done = pr
        if row_done is not None:
            emit_row_end(row_done, cq_rows[row_done])
            row_done = None
        pending = (r, k, mag)
    pr, pk, pmag = pending
    emit_cqt(pr, pk, pmag, cq_rows[pr])
    emit_row_end(pr, cq_rows[pr])


_PROGRAM_CACHE = {}


def _get_program():
    if "nc" not in _PROGRAM_CACHE:
        _PROGRAM_CACHE["nc"] = _build_program()
    return _PROGRAM_CACHE["nc"]


def kernel(audio):
    audio = np.asarray(audio, dtype=np.float32)
    assert audio.shape == (B, L), audio.shape

    # host data movement: reflect pad, shift by one sample, phase-major
    # transpose [B, 128, 4, NQ], dtype casts
    nsamp = 128 * 4 * NQ
    xp1 = np.zeros((B, nsamp), dtype=np.float32)
    xpad = np.pad(audio, ((0, 0), (PAD, PAD)), mode="reflect")
    xp1[:, : LP - 1] = xpad[:, 1:]
    # sample index 128*(4q+ph) + p  ->  [b, p, ph, q]
    a_t = xp1.reshape(B, NQ, 4, 128).transpose(0, 3, 2, 1)
    a16 = np.ascontiguousarray(a_t, dtype=ml_dtypes.bfloat16)
    a8 = np.ascontiguousarray(a_t, dtype=ml_dtypes.float8_e4m3)

    wc, ws, wq = _host_tables()
    # bf16 block: [p, a, f] with contraction chunk a, wc16[p,a,f]=wc[128a+p,f]
    wc16 = np.ascontiguousarray(
        wc[:, :FP_LO].reshape(16, 128, FP_LO).transpose(1, 0, 2),
        dtype=ml_dtypes.bfloat16)
    ws16 = np.ascontiguousarray(
        ws[:, :FP_LO].reshape(16, 128, FP_LO).transpose(1, 0, 2),
        dtype=ml_dtypes.bfloat16)
    # fp8 DoubleRow blocks: [p, s, u, fH], chunk a = 2s+u, fH = f-128
    wc8 = np.ascontiguousarray(
        wc[:, FP_LO:].reshape(8, 2, 128, FP_HI).transpose(2, 0, 1, 3),
        dtype=ml_dtypes.float8_e4m3)
    ws8 = np.ascontiguousarray(
        ws[:, FP_LO:].reshape(8, 2, 128, FP_HI).transpose(2, 0, 1, 3),
        dtype=ml_dtypes.float8_e4m3)
    # CQT weights: [p, a, k], contraction chunk a over the FP mag bins
    wqp = np.zeros((512, NBINS))
    wqp[:FP] = wq
    wq16 = np.ascontiguousarray(
        wqp.reshape(4, 128, NBINS).transpose(1, 0, 2), dtype=ml_dtypes.bfloat16)

    nc = _get_program()

    in_maps = []
    for c in range(NCORES):
        rows = slice(ROWS_PER_CORE * c, ROWS_PER_CORE * (c + 1))
        in_maps.append({
            "a16": np.ascontiguousarray(a16[rows]),
            "a8": np.ascontiguousarray(a8[rows]),
            "wc16": wc16, "ws16": ws16,
            "wc8": wc8, "ws8": ws8,
            "wq": wq16,
        })

    res = run_bass_kernel_spmd(nc, in_maps, core_ids=list(range(NCORES)))
    out = np.concatenate([res.results[c]["out"] for c in range(NCORES)], axis=0)
    return np.ascontiguousarray(out, dtype=np.float32)


# revision 6
# speedup vs baseline: 2.8182x; 1.4096x over previous
"""CQT extractor kernel for Trainium2 (8 NeuronCores, data-parallel over batch).

Per core (2 audio rows): STFT-as-matmul with Hermitian folding (1024-long
contraction instead of 2048), everything in bf16 — the fold adds run on
DVE/GPSIMD where 2-byte dtypes get fast-path throughput, the folded DFT
matmuls run at the PE's full bf16 rate.  Frequency bins >= 384 carry
negligible CQT weight and are truncated (384 = 3 blocks of 128 bins).
Magnitude via ACT Square / DVE add / ACT Sqrt, CQT projection in bf16,
log10 per row on ACT (Ln + Copy-with-scale) split into two column chunks so
only the last ~400 frames' worth sits after the final matmul.

Host side does only data movement (reflect pad, phase-major transpose
[128, 4, nq] of the shifted and the chunk-reversed signal, bf16 casts) and
constant table generation; all FLOPs run on device.
"""

import math
from contextlib import ExitStack

import ml_dtypes
import numpy as np

import concourse.tile as tile
from concourse import bacc, mybir
from concourse.bass_utils import run_bass_kernel_spmd

# ---- problem constants (hardcoded per contest rules) ----
B = 16
L = 1310720
SR = 22050
HOP = 512
NFFT = 2048
NBINS = 84
BPO = 12
FMIN = 27.5

NF = 1 + L // HOP            # 2561 frames
PAD = NFFT // 2              # 1024
LP = L + 2 * PAD             # 1312768 reflect-padded length

NCORES = 8
ROWS_PER_CORE = B // NCORES  # 2

FP = 384                     # frequency bins kept (of 1025), 3 blocks of 128
NBLK = FP // 128

# frame tiling: 6 uniform tiles of 428 frames; frames past NF-1 are computed
# on zero padding and never written out
NTILES = 6
T = 428
LNA = 5 * T                  # log10 chunk A covers tiles 0..4

# phase-major audio layout: chunk c = 4*q + ph, sample x[128*c + p]
NQ = 2576                    # q slots (covers chunk 4*2563+15 plus margin)

F32 = mybir.dt.float32
BF16 = mybir.dt.bfloat16
LOG10E = 1.0 / math.log(10.0)


def _host_tables():
    """Folded DFT matrices (1024 x FP) and CQT weights, float64."""
    j = np.arange(1024)
    n = (j + 1).astype(np.float64)   # contraction index j <-> sample n=j+1
    win = 0.5 * (1.0 - np.cos(2.0 * np.pi * n / NFFT))
    f = np.arange(FP, dtype=np.float64)
    ang = 2.0 * np.pi * np.outer(n, f) / NFFT
    wc = win[:, None] * np.cos(ang)
    ws = win[:, None] * np.sin(ang)
    wc[1023] *= 0.5           # n=1024 term is double-counted by the fold
    ws[1023] = 0.0
    sf = np.fft.rfftfreq(NFFT, 1.0 / SR)[:FP]
    cf = FMIN * 2.0 ** (np.arange(NBINS, dtype=np.float64) / BPO)
    wq = np.exp(-np.abs(sf[:, None] - cf[None, :]) / (0.1 * cf[None, :]))
    return wc, ws, wq


def _build_program():
    nc = bacc.Bacc("TRN2", target_bir_lowering=False, debug=False,
                   num_devices=NCORES)
    a16 = nc.dram_tensor("a16", [ROWS_PER_CORE, 128, 4, NQ], BF16,
                         kind="ExternalInput").ap()
    z16 = nc.dram_tensor("z16", [ROWS_PER_CORE, 128, 4, NQ], BF16,
                         kind="ExternalInput").ap()
    wcf = nc.dram_tensor("wcf", [128, 8, FP], BF16, kind="ExternalInput").ap()
    wsf = nc.dram_tensor("wsf", [128, 8, FP], BF16, kind="ExternalInput").ap()
    wq = nc.dram_tensor("wq", [128, NBLK, NBINS], BF16,
                        kind="ExternalInput").ap()
    out = nc.dram_tensor("out", [ROWS_PER_CORE, NBINS, NF], F32,
                         kind="ExternalOutput").ap()

    with tile.TileContext(nc) as tc:
        with ExitStack() as ctx:
            _emit(ctx, tc, a16, z16, wcf, wsf, wq, out)
    nc.compile()
    return nc


def _emit(ctx, tc, a16, z16, wcf, wsf, wq, out):
    nc = tc.nc
    SQ = mybir.ActivationFunctionType.Square
    SQRT = mybir.ActivationFunctionType.Sqrt
    LN = mybir.ActivationFunctionType.Ln

    consts = ctx.enter_context(tc.tile_pool(name="consts", bufs=1))
    a16p = ctx.enter_context(tc.tile_pool(name="a16p", bufs=3))
    z16p = ctx.enter_context(tc.tile_pool(name="z16p", bufs=3))
    eop = ctx.enter_context(tc.tile_pool(name="eop", bufs=2))
    sqp = ctx.enter_context(tc.tile_pool(name="sqp", bufs=6))
    magp = ctx.enter_context(tc.tile_pool(name="magp", bufs=3))
    cqp = ctx.enter_context(tc.tile_pool(name="cqp", bufs=2))
    outp = ctx.enter_context(tc.tile_pool(name="outp", bufs=4))
    ps_mm = ctx.enter_context(tc.tile_pool(name="ps_mm", bufs=5, space="PSUM"))
    ps_cq = ctx.enter_context(tc.tile_pool(name="ps_cq", bufs=2, space="PSUM"))

    # constants: DFT weights first (block 0's stationary gates the pipeline)
    wcf_sb = consts.tile([128, 8, FP], BF16, tag="wcf")
    wsf_sb = consts.tile([128, 8, FP], BF16, tag="wsf")
    wq_sb = consts.tile([128, NBLK, NBINS], BF16, tag="wq")
    nc.sync.dma_start(wcf_sb[:], wcf)
    nc.scalar.dma_start(wsf_sb[:], wsf)
    nc.gpsimd.dma_start(wq_sb[:], wq)
    lnbias = consts.tile([128, 1], F32, tag="lnbias")
    nc.gpsimd.memset(lnbias[:], 1e-10)

    def emit_tile(r, k):
        """DMA + fold + folded DFT matmuls + magnitude for one frame tile."""
        q0 = T * k
        a16_t = a16p.tile([128, 4, T + 4], BF16, tag="a16t")
        nc.sync.dma_start(a16_t[:], a16[r, :, :, q0: q0 + T + 4])
        z16_t = z16p.tile([128, 4, T + 4], BF16, tag="z16t")
        nc.scalar.dma_start(z16_t[:], z16[r, :, :, q0: q0 + T + 4])

        # fold: E[j,t] = x[512t+j+1] + x[512t+2047-j], O = diff; j-chunk a
        # reads x-chunk 4t+a (shifted copy) and x-chunk 4t+15-a (reversed)
        e16 = eop.tile([128, 8, T], BF16, tag="e16")
        o16 = eop.tile([128, 8, T], BF16, tag="o16")
        for a in range(8):
            d_ap = a16_t[:, a % 4, a // 4: a // 4 + T]
            r_ap = z16_t[:, (15 - a) % 4, (15 - a) // 4: (15 - a) // 4 + T]
            nc.vector.tensor_add(e16[:, a], d_ap, r_ap)
            eng = nc.vector if a < 4 else nc.gpsimd
            eng.tensor_sub(o16[:, a], d_ap, r_ap)

        mag = magp.tile([128, NBLK, T], BF16, tag="mag")
        for blk in range(NBLK):
            fb = 128 * blk
            ps_re = ps_mm.tile([128, T], F32, tag="mm")
            ps_im = ps_mm.tile([128, T], F32, tag="mm")
            for a in range(8):
                nc.tensor.matmul(ps_re[:, :T], wcf_sb[:, a, fb: fb + 128],
                                 e16[:, a], start=(a == 0), stop=(a == 7))
            for a in range(8):
                nc.tensor.matmul(ps_im[:, :T], wsf_sb[:, a, fb: fb + 128],
                                 o16[:, a], start=(a == 0), stop=(a == 7))
            sq = sqp.tile([128, T], BF16, tag="sq")
            nc.scalar.activation(sq[:], ps_re[:, :T], SQ)
            sq2 = sqp.tile([128, T], BF16, tag="sq2")
            nc.scalar.activation(sq2[:], ps_im[:, :T], SQ)
            ss = sqp.tile([128, T], BF16, tag="ss")
            nc.vector.tensor_add(ss[:], sq[:], sq2[:])
            nc.scalar.activation(mag[:, blk], ss[:], SQRT)
        return mag

    def emit_cqt(r, k, mag, cq_row):
        """CQT projection into PSUM, copy into the row accumulator."""
        q0 = T * k
        ps_c = ps_cq.tile([NBINS, T], F32, tag="cq")
        for a in range(NBLK):
            nc.tensor.matmul(ps_c[:, :T], wq_sb[:, a], mag[:, a],
                             start=(a == 0), stop=(a == NBLK - 1))
        V = min(T, NF - q0)
        nc.vector.tensor_copy(cq_row[:, q0: q0 + V], ps_c[:, :V])

    def emit_log(r, cq_row, c0, c1):
        """log10 = Ln then Copy-with-scale over columns [c0, c1), store."""
        out_t = outp.tile([NBINS, c1 - c0], F32, tag="outt",
                          name=f"outt{r}_{c0}")
        nc.scalar.activation(out_t[:], cq_row[:, c0: c1], LN,
                             bias=lnbias[:NBINS])
        nc.scalar.mul(out_t[:], out_t[:], LOG10E)
        nc.sync.dma_start(out[r, :, c0: c1], out_t[:])

    # software pipeline: PE order per slot is [dft k][cqt k-1] so the
    # magnitude drain of tile k-1 hides under tile k's matmuls; each row's
    # log10 chunk A fires once tiles 0..4 are reduced, chunk B at row end
    tiles = [(r, k) for r in range(ROWS_PER_CORE) for k in range(NTILES)]
    cq_rows = {r: cqp.tile([NBINS, NF], F32, tag="cqrow", name=f"cqrow{r}")
               for r in range(ROWS_PER_CORE)}
    pending = None
    for r, k in tiles:
        mag = emit_tile(r, k)
        if pending is not None:
            pr, pk, pmag = pending
            emit_cqt(pr, pk, pmag, cq_rows[pr])
            if pk == NTILES - 2:
                emit_log(pr, cq_rows[pr], 0, LNA)
            elif pk == NTILES - 1:
                emit_log(pr, cq_rows[pr], LNA, NF)
        pending = (r, k, mag)
    pr, pk, pmag = pending
    emit_cqt(pr, pk, pmag, cq_rows[pr])
    emit_log(pr, cq_rows[pr], LNA, NF)


_PROGRAM_CACHE = {}


def _get_program():
    if "nc" not in _PROGRAM_CACHE:
        _PROGRAM_CACHE["nc"] = _build_program()
    return _PROGRAM_CACHE["nc"]


def kernel(audio):
    audio = np.asarray(audio, dtype=np.float32)
    assert audio.shape == (B, L), audio.shape

    # host data movement: reflect pad, then two phase-major transposed
    # copies — a16 shifted by one sample, z16 reversed within each chunk
    nsamp = 128 * 4 * NQ
    xpad = np.pad(audio, ((0, 0), (PAD, PAD)), mode="reflect")
    xp1 = np.zeros((B, nsamp), dtype=np.float32)
    xp1[:, : LP - 1] = xpad[:, 1:]
    a_t = xp1.reshape(B, NQ, 4, 128).transpose(0, 3, 2, 1)
    a16 = np.ascontiguousarray(a_t, dtype=ml_dtypes.bfloat16)
    xpz = np.zeros((B, nsamp), dtype=np.float32)
    xpz[:, :LP] = xpad
    z_t = xpz.reshape(B, nsamp // 128, 128)[:, :, ::-1]
    z_t = z_t.reshape(B, NQ, 4, 128).transpose(0, 3, 2, 1)
    z16 = np.ascontiguousarray(z_t, dtype=ml_dtypes.bfloat16)

    wc, ws, wq = _host_tables()
    # folded weights: [p, a, f] with contraction chunk a, wcf[p,a,f]=wc[128a+p,f]
    wcf = np.ascontiguousarray(
        wc.reshape(8, 128, FP).transpose(1, 0, 2), dtype=ml_dtypes.bfloat16)
    wsf = np.ascontiguousarray(
        ws.reshape(8, 128, FP).transpose(1, 0, 2), dtype=ml_dtypes.bfloat16)
    # CQT weights: [p, a, k], contraction chunk a over the FP mag bins
    wq16 = np.ascontiguousarray(
        wq.reshape(NBLK, 128, NBINS).transpose(1, 0, 2),
        dtype=ml_dtypes.bfloat16)

    nc = _get_program()

    in_maps = []
    for c in range(NCORES):
        rows = slice(ROWS_PER_CORE * c, ROWS_PER_CORE * (c + 1))
        in_maps.append({
            "a16": np.ascontiguousarray(a16[rows]),
            "z16": np.ascontiguousarray(z16[rows]),
            "wcf": wcf, "wsf": wsf, "wq": wq16,
        })

    res = run_bass_kernel_spmd(nc, in_maps, core_ids=list(range(NCORES)))
    out = np.concatenate([res.results[c]["out"] for c in range(NCORES)], axis=0)
    return np.ascontiguousarray(out, dtype=np.float32)
